# revision 29
# baseline (speedup 1.0000x reference)
"""Bass/Tile Trainium2 kernel for a 2-layer dense multi-head GAT over a batch
of B=8 independent subgraphs (2048 nodes each, equal contiguous segments).

Sharding: one subgraph per NeuronCore (8 cores), parameters replicated.

Algorithm (per core / subgraph, per attention layer):
  scores are rank-1:  e_ij = leaky_relu(s1_i + s2_j),  s1 = h@a1, s2 = h@a2.
  exp(leaky_relu(t)) is separable through the sign mask M_ij = [s1_i+s2_j>=0]:
      p_ij = M_ij e^{s1_i} e^{s2_j} + (1-M_ij) e^{a s1_i} e^{a s2_j}
  so softmax(e) @ h needs NO N^2 exp work:
      num_i = g_i (M @ u)_i + (vtot - (M @ v))_i          (e^{a s1} cancels in
      u_j = e^{s2_j} [h_j|1],  v_j = e^{a s2_j} [h_j|1],   the Z ratio; g =
      out_i = num_i[:64] / num_i[64]                       e^{(1-a) s1})
  The N^2 work is one DVE compare pass per j-chunk (full 2048-wide bf16 mask
  rows, 4x DVE mode) plus bf16 mask matmuls (single bf16 stream; the 2e-2
  tolerance has plenty of headroom for bf16 rounding of u/v).
"""

from contextlib import ExitStack

import numpy as np

import concourse.bass as bass
import concourse.tile as tile
from concourse import bacc, mybir
from concourse.masks import make_identity

FP = mybir.dt.float32
BF = mybir.dt.bfloat16
AF = mybir.ActivationFunctionType
OP = mybir.AluOpType

B = 8
N = 2048
D = 64
H = 4
ALPHA = 0.2
P = 128
NCH = N // P  # 16 chunks of 128 nodes
DEXT = D + 1  # h plus ones column
W2 = 2 * DEXT  # u|v stream width


def _attention(nc, pools, scratch, s12, s1b, hext, out_cb, masks=None,
               emit=None, bg=None):
    """Dense-GAT attention layer: out = softmax(lrelu(s1_i+s2_j)) @ h.

    s12:  [P, NCH, 2] SBUF f32 (s1|s2 in node-chunk column layout)
    s1b:  [P, N] SBUF bf16 (s1 replicated across partitions, free dim = node)
    hext: [P, NCH, DEXT] SBUF bf16 (h natural, col D == 1.0)
    out_cb(onorm, rz, q): consumes quarter q of the un-normalized [P, NCH,
        DEXT] nsum plus per-chunk reciprocal Z.
    masks/emit: optionally pre-populated mask dict + its emitter (cross-layer
        mask prefetch); bg(q) is called after each quarter to stage the next
        layer's prep/masks into the engine queues.
    """
    const, prep, mask_pool, wide, small, psA, psaux = pools

    # --- exponentials (split per s12 group so chunk-0's u/v tiles don't wait
    # for all 16 s12 columns) ---
    es2 = prep.tile([P, NCH], FP, tag="es2")
    es02 = prep.tile([P, NCH], FP, tag="es02")
    g = prep.tile([P, NCH], FP, tag="g")
    for cg in range(4):
        gs = slice(cg * 4, (cg + 1) * 4)
        nc.scalar.activation(es2[:, gs], s12[:, gs, 1], AF.Exp)
        nc.scalar.activation(es02[:, gs], s12[:, gs, 1], AF.Exp, scale=ALPHA)
    nc.scalar.activation(g, s12[:, :, 0], AF.Exp, scale=1.0 - ALPHA)

    # --- masks: one [P, 2048] bf16 row per j-chunk (16 total), consumed by
    # every quarter; emitted with lookahead so DVE stays ahead of the PE ---
    if masks is None:
        masks = {}
    if emit is None:
        def emit(jc, _m=masks, _s12=s12, _s1b=s1b):
            if jc >= NCH or jc in _m:
                return
            mt = mask_pool.tile([P, N], BF, tag="mt", name=f"mt{jc}")
            nc.vector.tensor_scalar(mt, _s1b, _s12[:, jc, 1:2], 0.0,
                                    OP.add, OP.is_ge)
            _m[jc] = mt
    mask_tiles = masks
    emit_mask = emit

    # --- u | -v tiles, single bf16 stream (hext is already bf16, 4x DVE);
    # the v half is negated in the same op via the second scalar slot ---
    LOOKAHEAD = 4
    uv = prep.tile([P, NCH, W2], BF, tag="uv")
    for c in range(NCH):
        nc.vector.tensor_scalar(uv[:, c, 0:DEXT], hext[:, c, :],
                                es2[:, c:c + 1], None, OP.mult)
        nc.vector.tensor_scalar(uv[:, c, DEXT:], hext[:, c, :],
                                es02[:, c:c + 1], -1.0, OP.mult, OP.mult)
    for jc in range(LOOKAHEAD):
        emit_mask(jc)

    # --- vtot row [0...0 | sum_j v_j] (bf16 hi+res rows): negate (the
    # stream holds -v) and split into hi+res single-partition rows. The DVE
    # ops sit before the bulk of mask emission so the quarter-0 seeds are
    # never stuck behind the mask queue; seeding via two K=1 matmuls avoids
    # any DMA on this path. ---
    ones_col_bf = scratch["ones_col_bf"]
    ones_row_bf = scratch["ones_row_bf"]
    vt_ps = psaux.tile([1, DEXT], FP, tag="aux")
    for c in range(NCH):
        nc.tensor.matmul(vt_ps, ones_col_bf, uv[:, c, DEXT:],
                         start=(c == 0), stop=(c == NCH - 1))
    vrow_bf = prep.tile([1, W2], BF, tag="vrow_bf")
    vres = prep.tile([1, W2], BF, tag="vres")
    nc.vector.memset(vrow_bf[:, 0:DEXT], 0.0)
    nc.vector.memset(vres[:, 0:DEXT], 0.0)
    nc.vector.tensor_scalar(vrow_bf[:, DEXT:], vt_ps, -1.0, None, OP.mult)
    nc.vector.scalar_tensor_tensor(vres[:, DEXT:], vt_ps, -1.0,
                                   vrow_bf[:, DEXT:], OP.mult, OP.subtract)

    # --- masked attention matmuls + per-chunk epilogue, quarter-pipelined ---
    nsum_w = wide.tile([P, NCH, DEXT], FP, tag="nsum")
    for q in range(4):  # quarters of the i (destination-node) axis
        A = [psA.tile([P, W2], FP, tag="A", name=f"A{q}_{il}")
             for il in range(4)]
        for jc in range(NCH):
            mt = mask_tiles[jc]
            if q == 0:
                emit_mask(jc + LOOKAHEAD)
            for il in range(4):
                sl = mt[:, q * 512 + il * P:q * 512 + (il + 1) * P]
                nc.tensor.matmul(A[il], sl, uv[:, jc, :],
                                 start=(jc == 0), stop=False)
        # seed vtot last — PSUM accumulation is order-insensitive
        for il in range(4):
            nc.tensor.matmul(A[il], ones_row_bf[0:1, :], vrow_bf, start=False,
                             stop=False)
            nc.tensor.matmul(A[il], ones_row_bf[0:1, :], vres, start=False,
                             stop=True)
        qs = slice(q * 4, (q + 1) * 4)
        for il in range(4):
            ic = q * 4 + il
            # evacuate A to SBUF on ACT (DVE may read only one PSUM operand),
            # then nsum = g * (M@u) + (vtot - M@v) all-SBUF at 2x DVE rate
            E = small.tile([P, W2], FP, tag="E")
            nc.scalar.copy(E, A[il])
            nc.vector.scalar_tensor_tensor(nsum_w[:, ic, :], E[:, 0:DEXT],
                                           g[:, ic:ic + 1], E[:, DEXT:],
                                           OP.mult, OP.add)
        # reciprocal of this quarter's Z and hand off
        rz = small.tile([P, 4], FP, tag="rz")
        nc.vector.reciprocal(rz, nsum_w[:, qs, D])
        out_cb(nsum_w, rz, q)
        if bg is not None:
            bg(q)


def _elu_q(nc, pools, nsum_w, rz, q, xc_dst, dt):
    """normalize + elu over quarter q; writes elu(nsum/Z) into xc_dst
    ([P, 4, D] slice view) with dtype dt."""
    const, prep, mask_pool, wide, small, psA, psaux = pools
    onorm = wide.tile([P, 4, D], dt, tag=f"onorm{dt}", name=f"onorm{q}")
    for k in range(4):
        ic = q * 4 + k
        nc.vector.tensor_scalar(onorm[:, k, :], nsum_w[:, ic, 0:D],
                                rz[:, k:k + 1], None, OP.mult)
    # elu(x) = (min(e^x, 1) - 1) + max(x, 0); e^x finite since x <= ~20
    e = wide.tile([P, 4, D], dt, tag=f"elu_e{dt}", name=f"elu_e{q}")
    nc.scalar.activation(e, onorm, AF.Exp)
    r = wide.tile([P, 4, D], dt, tag=f"elu_r{dt}", name=f"elu_r{q}")
    nc.vector.tensor_scalar(r, onorm, 0.0, -1.0, OP.max, OP.add)
    nc.vector.scalar_tensor_tensor(xc_dst, e, 1.0, r, OP.min, OP.add)
    return onorm


def build_kernel():
    nc = bacc.Bacc("TRN2", target_bir_lowering=False, debug=False,
                   num_devices=B)

    x = nc.dram_tensor("x", [N, D], FP, kind="ExternalInput")
    W_heads = nc.dram_tensor("W_heads", [H, D, D], FP, kind="ExternalInput")
    a_heads = nc.dram_tensor("a_heads", [H, 2 * D], FP, kind="ExternalInput")
    W_out = nc.dram_tensor("W_out", [H * D, D], FP, kind="ExternalInput")
    a_out = nc.dram_tensor("a_out", [2 * D], FP, kind="ExternalInput")
    out = nc.dram_tensor("out", [N, D], FP, kind="ExternalOutput")

    with tile.TileContext(nc) as tc, ExitStack() as ctx:
        const = ctx.enter_context(tc.tile_pool(name="const", bufs=1))
        prep = ctx.enter_context(tc.tile_pool(name="prep", bufs=3))
        mask_pool = ctx.enter_context(tc.tile_pool(name="mask", bufs=20))
        wide = ctx.enter_context(tc.tile_pool(name="wide", bufs=3))
        small = ctx.enter_context(tc.tile_pool(name="small", bufs=6))
        psA = ctx.enter_context(tc.tile_pool(name="psA", bufs=5, space="PSUM"))
        psaux = ctx.enter_context(tc.tile_pool(name="psaux", bufs=3, space="PSUM"))
        pools = (const, prep, mask_pool, wide, small, psA, psaux)

        ident = const.tile([P, P], FP)
        make_identity(nc, ident)
        ident_bf = const.tile([P, P], BF)
        nc.vector.tensor_copy(ident_bf, ident)
        ones128 = const.tile([P, P], FP)
        nc.vector.memset(ones128, 1.0)
        ones_col_bf = const.tile([P, 1], BF)
        nc.vector.memset(ones_col_bf, 1.0)
        ones_row_bf = const.tile([2, P], BF)
        nc.vector.memset(ones_row_bf, 1.0)
        scratch = {"ones128": ones128, "ones_col_bf": ones_col_bf,
                   "ones_row_bf": ones_row_bf}

        # ---- load inputs (x in 4 pieces so transposes start early) ----
        x_sb = const.tile([P, NCH, D], FP)
        x_r = x.rearrange("(c p) d -> p c d", p=P)
        for r4 in range(4):
            nc.sync.dma_start(out=x_sb[:, r4 * 4:(r4 + 1) * 4, :],
                              in_=x_r[:, r4 * 4:(r4 + 1) * 4, :])
        Wh = const.tile([64, H, D], FP)
        nc.sync.dma_start(out=Wh, in_=W_heads.rearrange("h k d -> k h d"))
        Wh_bf = const.tile([64, H, D], BF)
        nc.vector.tensor_copy(Wh_bf, Wh)
        WhT = const.tile([64, H, D], FP)
        nc.sync.dma_start(out=WhT, in_=W_heads.rearrange("h k d -> d h k"))
        a_sb = const.tile([64, H, 2], FP)
        nc.sync.dma_start(out=a_sb, in_=a_heads.rearrange("h (t k) -> k h t", t=2))
        Wo = const.tile([P, 2, D], FP)
        nc.sync.dma_start(out=Wo, in_=W_out.rearrange("(c k) d -> k c d", k=P))
        Wo_bf = const.tile([P, 2, D], BF)
        nc.vector.tensor_copy(Wo_bf, Wo)
        WoT = const.tile([64, 2, P], FP)
        nc.sync.dma_start(out=WoT, in_=W_out.rearrange("(c k) d -> d c k", k=P))
        ao = const.tile([64, 2], FP)
        nc.sync.dma_start(out=ao, in_=a_out.rearrange("(t k) -> k t", t=2))

        # ---- xT via PE transposes; bf16 shadow (in pieces) ----
        xT = const.tile([64, N], FP)
        for cp in range(8):  # transpose pairs: one evac per two chunks
            tp = psaux.tile([64, 2, P], FP, tag="aux")
            for k in range(2):
                nc.tensor.transpose(tp[:, k, :], x_sb[:, 2 * cp + k, :], ident)
            # alternate evac engines so ACT is free for the head-0 prep chain
            if cp % 2 == 0:
                nc.vector.tensor_copy(xT[:, 2 * cp * P:(2 * cp + 2) * P], tp)
            else:
                nc.scalar.copy(xT[:, 2 * cp * P:(2 * cp + 2) * P], tp)
        xT_bf = const.tile([64, N], BF)
        for r in range(4):
            nc.vector.tensor_copy(xT_bf[:, r * 512:(r + 1) * 512],
                                  xT[:, r * 512:(r + 1) * 512])

        # all heads' wa = W_h @ [a1|a2] upfront (re-association: s = x @ wa);
        # only needs the parameter DMAs, so it fills the startup bubble
        wa_all = const.tile([64, H, 2], FP)
        for h in range(H):
            wap = psaux.tile([64, 2], FP, tag="aux", name=f"wap{h}")
            nc.tensor.matmul(wap, WhT[:, h, :], a_sb[:, h, :], start=True,
                             stop=True)
            nc.scalar.copy(wa_all[:, h, :], wap)

        # layer-2 score weights too — they only need the parameter DMAs
        wa2 = const.tile([P, 2, 2], FP)
        wa2_bf = const.tile([P, 2, 2], BF)
        for kc in range(2):
            wap = psaux.tile([P, 2], FP, tag="aux", name=f"wap2_{kc}")
            nc.tensor.matmul(wap, WoT[:, kc, :], ao, start=True, stop=True)
            nc.scalar.copy(wa2[:, kc, :], wap)
            nc.vector.tensor_copy(wa2_bf[:, kc, :], wa2[:, kc, :])
        wa1b2 = const.tile([P, 2, P], BF)
        for kc in range(2):
            nc.vector.tensor_scalar(wa1b2[:, kc, :], ones128, wa2[:, kc, 0:1],
                                    None, OP.mult)

        # ---- layer 1: four heads -> xc01/xc23 (bf16; split so the layer-2
        # transposes of head-pair 0/1 need not wait for heads 2/3) ----
        xc01 = const.tile([P, NCH, 2, D], BF)
        xc23 = const.tile([P, NCH, 2, D], BF)

        heads = {}

        def l1_prep_scores(h):
            # s12 columns + s1b: everything the masks depend on
            wa = wa_all[:, h, :]
            s12 = prep.tile([P, NCH, 2], FP, tag="s12", name=f"s12_{h}")
            for cg in range(4):
                sp = psaux.tile([P, 8], FP, tag="aux", name=f"sp{h}_{cg}")
                for k in range(4):
                    c = cg * 4 + k
                    nc.tensor.matmul(sp[:, 2 * k:2 * k + 2],
                                     xT[:, c * P:(c + 1) * P], wa,
                                     start=True, stop=True)
                nc.scalar.copy(s12[:, cg * 4:(cg + 1) * 4, :], sp)

            # s1b (bf16, mask input only): s1 row replicated via ones x wa1
            wa1b = prep.tile([64, P], BF, tag="wa1b", name=f"wa1b_{h}")
            nc.vector.tensor_scalar(wa1b, ones128[0:64, :], wa[:, 0:1], None,
                                    OP.mult)
            s1b = prep.tile([P, N], BF, tag="s1b", name=f"s1b_{h}")
            for r in range(8):
                ps = psaux.tile([P, 256], FP, tag="aux")
                nc.tensor.matmul(ps, wa1b, xT_bf[:, r * 256:(r + 1) * 256],
                                 start=True, stop=True)
                nc.scalar.copy(s1b[:, r * 256:(r + 1) * 256], ps)
            heads[h] = {"s12": s12, "s1b": s1b, "masks": {}}

        def l1_prep_hext(h):
            # h natural (+ones col), bf16 (bf16 moving operand: 1 cyc/row);
            # 4 chunks per PSUM tile so one ACT copy evacuates 4 matmuls
            hext = prep.tile([P, NCH, DEXT], BF, tag="hext", name=f"hext_{h}")
            nc.vector.memset(hext[:, :, D], 1.0)
            for cg in range(4):
                hp = psaux.tile([P, 4, D], FP, tag="aux")
                for k in range(4):
                    c = cg * 4 + k
                    nc.tensor.matmul(hp[:, k, :], xT_bf[:, c * P:(c + 1) * P],
                                     Wh_bf[:, h, :], start=True, stop=True)
                nc.scalar.copy(hext[:, cg * 4:(cg + 1) * 4, 0:D], hp)
            heads[h]["hext"] = hext

        def l1_emitter(h):
            def em(jc):
                he = heads[h]
                if jc >= NCH or jc in he["masks"]:
                    return
                mt = mask_pool.tile([P, N], BF, tag="mt", name=f"m{h}_{jc}")
                nc.vector.tensor_scalar(mt, he["s1b"],
                                        he["s12"][:, jc, 1:2], 0.0,
                                        OP.add, OP.is_ge)
                he["masks"][jc] = mt
            return em

        l1_prep_scores(0)
        l1_prep_hext(0)
        for h in range(H):
            def l1_out(nsum_w, rz, q, h=h):
                xc = xc01 if h < 2 else xc23
                _elu_q(nc, pools, nsum_w, rz, q,
                       xc[:, q * 4:(q + 1) * 4, h % 2, :], BF)

            def bg(q, h=h):
                # stage the next head's prep + masks into the engine queues
                # while this head's quarters run (keeps the next head's
                # quarter 0 from being paced by its mask generation)
                nh = h + 1
                if nh >= H:
                    return
                nem = l1_emitter(nh)
                if q == 0:
                    l1_prep_scores(nh)
                elif q == 1:
                    l1_prep_hext(nh)
                    for jc in range(4):
                        nem(jc)
                elif q == 2:
                    for jc in range(4, 8):
                        nem(jc)
                elif q == 3:
                    for jc in range(8, NCH):
                        nem(jc)

            he = heads[h]
            _attention(nc, pools, scratch, he["s12"], he["s1b"], he["hext"],
                       l1_out, masks=he["masks"], emit=l1_emitter(h), bg=bg)

        # ---- transpose xc -> xcT_bf [P, 2, N] (feature-major) ----
        xcT_bf = const.tile([P, 2, N], BF)
        for kc, xc in ((0, xc01), (1, xc23)):
            for cp in range(8):  # transpose pairs: one evac per two chunks
                tp = psaux.tile([P, 2, P], BF, tag="aux")
                for k in range(2):
                    nc.tensor.transpose(tp[:, k, :], xc[:, 2 * cp + k, :, :],
                                        ident_bf)
                # alternate evac engines: ACT is busy with the last heads'
                # epilogue work in this region
                if (cp + kc) % 2 == 0:
                    nc.vector.tensor_copy(
                        xcT_bf[:, kc, 2 * cp * P:(2 * cp + 2) * P], tp)
                else:
                    nc.scalar.copy(
                        xcT_bf[:, kc, 2 * cp * P:(2 * cp + 2) * P], tp)

        # ---- layer 2 projections (all from bf16 xcT) ----
        s12_2 = prep.tile([P, NCH, 2], FP, tag="s12")
        for cg in range(4):
            sp = psaux.tile([P, 8], FP, tag="aux", name=f"sp2_{cg}")
            for k in range(4):
                c = cg * 4 + k
                for kc in range(2):
                    nc.tensor.matmul(sp[:, 2 * k:2 * k + 2],
                                     xcT_bf[:, kc, c * P:(c + 1) * P],
                                     wa2_bf[:, kc, :],
                                     start=(kc == 0), stop=(kc == 1))
            nc.scalar.copy(s12_2[:, cg * 4:(cg + 1) * 4, :], sp)

        s1b_2 = prep.tile([P, N], BF, tag="s1b")
        for r in range(8):
            ps = psaux.tile([P, 256], FP, tag="aux")
            for kc in range(2):
                nc.tensor.matmul(ps, wa1b2[:, kc, :],
                                 xcT_bf[:, kc, r * 256:(r + 1) * 256],
                                 start=(kc == 0), stop=(kc == 1))
            nc.scalar.copy(s1b_2[:, r * 256:(r + 1) * 256], ps)

        h2ext = prep.tile([P, NCH, DEXT], BF, tag="hext")
        nc.vector.memset(h2ext[:, :, D], 1.0)
        for cg in range(4):
            hp = psaux.tile([P, 4, D], FP, tag="aux")
            for k in range(4):
                c = cg * 4 + k
                for kc in range(2):
                    nc.tensor.matmul(hp[:, k, :],
                                     xcT_bf[:, kc, c * P:(c + 1) * P],
                                     Wo_bf[:, kc, :], start=(kc == 0),
                                     stop=(kc == 1))
            nc.scalar.copy(h2ext[:, cg * 4:(cg + 1) * 4, 0:D], hp)

        # ---- layer 2 attention + elu + log_softmax -> out ----
        out_w = const.tile([P, NCH, D], FP)

        out_r = out.rearrange("(c p) d -> p c d", p=P)
        o2_all = const.tile([P, NCH, D], FP)
        esum_all = const.tile([P, NCH], FP)

        def l2_out(nsum_w, rz, q):
            # per quarter: normalize + elu (fp32 here: o2 feeds log_softmax
            # directly) + raw exp-sum (elu output is <= ~20, so exp is
            # fp32-safe without max subtraction), then the full log-softmax
            # tail + output DMA for this quarter (Exp and Ln share the
            # natural_log_exp_and_others ACT table, so no table swaps)
            qs = slice(q * 4, (q + 1) * 4)
            o2 = o2_all[:, qs, :]
            _elu_q(nc, pools, nsum_w, rz, q, o2, FP)
            escr = wide.tile([P, 4, D], FP, tag="escr", name=f"escr{q}")
            nc.scalar.activation(escr, o2, AF.Exp)
            nc.vector.tensor_reduce(esum_all[:, qs], escr,
                                    mybir.AxisListType.X, OP.add)
            lseq = wide.tile([P, 4], FP, tag="lse", name=f"lse{q}")
            nc.scalar.activation(lseq, esum_all[:, qs], AF.Ln)
            lse_b = bass.AP(tensor=lseq.tensor, offset=lseq.offset,
                            ap=[lseq.ap[0], lseq.ap[1], [0, D]])
            nc.vector.tensor_tensor(out_w[:, qs, :], o2, lse_b, OP.subtract)
            nc.sync.dma_start(out=out_r[:, qs, :], in_=out_w[:, qs, :])

        _attention(nc, pools, scratch, s12_2, s1b_2, h2ext, l2_out)

    nc.compile()
    return nc


_NC_CACHE = {}


def _make_runner(nc):
    """Build a cached sharded executable (run_bass_kernel_spmd re-traces
    jax.jit on every call; this jits once and reuses)."""
    import jax
    from jax.sharding import Mesh, PartitionSpec
    try:
        from jax.experimental.shard_map import shard_map
    except ImportError:
        from jax.shard_map import shard_map
    import concourse.mybir as mb
    from concourse import bass2jax

    bass2jax.install_neuronx_cc_hook()

    part_name = nc.partition_id_tensor.name if nc.partition_id_tensor else None
    in_names, out_names, out_avals = [], [], []
    for alloc in nc.m.functions[0].allocations:
        if not isinstance(alloc, mb.MemoryLocationSet):
            continue
        name = alloc.memorylocations[0].name
        if alloc.kind == "ExternalInput":
            if name != part_name:
                in_names.append(name)
        elif alloc.kind == "ExternalOutput":
            out_names.append(name)
            out_avals.append(jax.core.ShapedArray(
                tuple(alloc.tensor_shape), mb.dt.np(alloc.dtype)))
    n_params = len(in_names)
    all_names = in_names + out_names
    if part_name is not None:
        all_names = all_names + [part_name]

    def _body(*args):
        operands = list(args)
        if part_name is not None:
            operands.append(bass2jax.partition_id_tensor())
        return tuple(bass2jax._bass_exec_p.bind(
            *operands, out_avals=tuple(out_avals), in_names=tuple(all_names),
            out_names=tuple(out_names), lowering_input_output_aliases=(),
            sim_require_finite=True, sim_require_nnan=True, nc=nc))

    devices = jax.devices()[:B]
    mesh = Mesh(np.asarray(devices), ("core",))
    n_outs = len(out_names)
    sharded = jax.jit(
        shard_map(_body, mesh=mesh,
                  in_specs=(PartitionSpec("core"),) * (n_params + n_outs),
                  out_specs=(PartitionSpec("core"),) * n_outs,
                  check_rep=False),
        donate_argnums=tuple(range(n_params, n_params + n_outs)),
        keep_unused=True)

    def run(in_maps):
        concat_in = [
            np.concatenate([np.asarray(in_maps[c][nm])[None] for c in range(B)],
                           axis=0).reshape(B * in_maps[0][nm].shape[0],
                                           *in_maps[0][nm].shape[1:])
            for nm in in_names
        ]
        concat_zeros = [
            np.zeros((B * av.shape[0], *av.shape[1:]), av.dtype)
            for av in out_avals
        ]
        out_arrs = sharded(*concat_in, *concat_zeros)
        return [
            {nm: np.asarray(out_arrs[i]).reshape(B, *out_avals[i].shape)[c]
             for i, nm in enumerate(out_names)}
            for c in range(B)
        ]

    return run


def kernel(**inputs):
    h_states = np.ascontiguousarray(np.asarray(inputs["h_states"], dtype=np.float32))
    W_heads = np.ascontiguousarray(np.asarray(inputs["W_heads"], dtype=np.float32))
    a_heads = np.ascontiguousarray(np.asarray(inputs["a_heads"], dtype=np.float32))
    W_out = np.ascontiguousarray(np.asarray(inputs["W_out"], dtype=np.float32))
    a_out = np.ascontiguousarray(np.asarray(inputs["a_out"], dtype=np.float32))

    if "nc" not in _NC_CACHE:
        _NC_CACHE["nc"] = build_kernel()
        _NC_CACHE["run"] = _make_runner(_NC_CACHE["nc"])

    xs = h_states.reshape(B, N, D)
    in_maps = [
        {"x": xs[c], "W_heads": W_heads, "a_heads": a_heads,
         "W_out": W_out, "a_out": a_out}
        for c in range(B)
    ]
    results = _NC_CACHE["run"](in_maps)
    return np.concatenate([results[c]["out"] for c in range(B)], axis=0)


if __name__ == "__main__":
    # smoke test (self-contained: random inputs, shape/dtype check only)
    rng = np.random.default_rng(0)
    inputs = {
        "h_states": rng.standard_normal((B * N, D)).astype(np.float32),
        "W_heads": rng.standard_normal((H, D, D)).astype(np.float32) * 0.18,
        "a_heads": rng.standard_normal((H, 2 * D)).astype(np.float32) * 0.18,
        "W_out": rng.standard_normal((H * D, D)).astype(np.float32) * 0.09,
        "a_out": rng.standard_normal((2 * D,)).astype(np.float32) * 0.18,
        "seq_start_end": (np.arange(B, dtype=np.int32)[:, None] * N
                          + np.array([0, N], dtype=np.int32)[None, :]),
    }
    got = kernel(**inputs)
    print("kernel output", got.shape, got.dtype)
